# revision 1
# baseline (speedup 1.0000x reference)
"""Trainium2 Bass kernel: causal self-attention (B=2, T=2048, C=1024, H=16, Dh=64).

Sharding: 8 cores = 2 (batch) x 4 (head groups of 4 heads).  Each core gets
x[b] plus the W_qkv rows / W_proj columns for its heads, computes the full
attention + a partial output projection for its batch, and the host sums the
4 partials per batch (tensor-parallel unshard).

All matmuls run in bf16 with f32 PSUM accumulation.  x is passed transposed
(xT = x[b].T) so that:
  qT, kT = Wq @ xT, Wk @ xT     (head dim on partitions)  -- no transposes
  v      = xT.T @ WvT           (natural [T, d] layout)
  S^T    = kT_h(tile).T @ qT_h  ([k, q] layout, 128x512 blocks)
  exp on ScalarE (logits are bounded, no max pass needed); causal masking by
  computing only the live columns of each block (diagonal blocks truncate
  their dead leading columns in the S matmul, the exp, and the y matmul) plus
  one multiplicative [128,128] triangle mask on the diagonal subtile; row
  sums via a ones column appended to V (so P@[V|1] accumulates y^T and the
  softmax denominators in one PSUM tile); the 1/sum normalization applied on
  eviction using a PE-broadcast reciprocal row.
  out_partial = y^T.T @ WpT   (f32, DMA'd out).

ScalarE runs nothing but Exp during the attention phase -- any other
activation function (even Copy) can trigger a ~2.7us ACT table reload.
"""
import sys
import types

import numpy as np
import ml_dtypes

_BF16 = ml_dtypes.bfloat16


def _install_ntff_hook():
    """Provide antenv.axon_hooks so run_bass_kernel_spmd(trace=True) works."""
    if "antenv.axon_hooks" in sys.modules:
        return
    mod = types.ModuleType("antenv.axon_hooks")
    mod._hook = None

    def set_axon_ntff_profile_hook(h):
        mod._hook = h

    def get_axon_ntff_profile_hook():
        return mod._hook

    mod.set_axon_ntff_profile_hook = set_axon_ntff_profile_hook
    mod.get_axon_ntff_profile_hook = get_axon_ntff_profile_hook
    sys.modules["antenv.axon_hooks"] = mod
    try:
        import antenv

        antenv.axon_hooks = mod
    except Exception:
        pass
    try:
        from trn_agent_boot.trn_boot import _ntff_profile_via_ctypes

        mod.set_axon_ntff_profile_hook(
            _ntff_profile_via_ctypes("/opt/axon/libaxon_pjrt.so")
        )
    except Exception:
        pass


_install_ntff_hook()

import concourse.bacc as bacc
import concourse.mybir as mybir
from concourse import bass_utils
from concourse.tile import TileContext

# no network bucket in this container; keep artifacts local
bass_utils.upload_artifacts = lambda tmpdir: tmpdir

BF16 = mybir.dt.bfloat16
F32 = mybir.dt.float32

B, T, C = 2, 2048, 1024
H, D = 16, 64
HL = 4            # heads per core
OL = HL * D       # 256 local qkv output dim
P = 128
KC = C // P       # 8 contraction chunks
NQT = T // P      # 16 q/k 128-tiles
NQC = T // 512    # 4 q 512-chunks
VA = D + 1        # v columns per head incl. ones column (65)

_nc_cache = None


def _build_nc():
    nc = bacc.Bacc("TRN2", target_bir_lowering=False, debug=False, num_devices=8)

    # all inputs arrive pre-arranged in SBUF-image layout [128, X] so every
    # input DMA moves multi-KB contiguous runs per partition row.
    xT = nc.declare_dram_parameter("xT", [P, KC * T], BF16, isOutput=False)
    wqT = nc.declare_dram_parameter("wqT", [P, KC * OL], BF16, isOutput=False)
    wkT = nc.declare_dram_parameter("wkT", [P, KC * OL], BF16, isOutput=False)
    wvT = nc.declare_dram_parameter("wvT", [P, KC * OL], BF16, isOutput=False)
    wpT = nc.declare_dram_parameter("wpT", [P, 2 * C], BF16, isOutput=False)
    mk = nc.declare_dram_parameter("mask_tri", [P, P], BF16, isOutput=False)
    out = nc.declare_dram_parameter("out", [T, C], F32, isOutput=True)

    Exp = mybir.ActivationFunctionType.Exp

    with TileContext(nc) as tc:
        with tc.tile_pool(name="const", bufs=1) as const, \
             tc.tile_pool(name="misc", bufs=3) as misc, \
             tc.tile_pool(name="att", bufs=8) as att, \
             tc.tile_pool(name="outp", bufs=6) as outp:
            xT_sb = const.tile([P, KC * T], BF16, name="xT_sb")
            wq_sb = const.tile([P, KC * OL], BF16, name="wq_sb")
            wk_sb = const.tile([P, KC * OL], BF16, name="wk_sb")
            wv_sb = const.tile([P, KC * OL], BF16, name="wv_sb")
            wp_sb = const.tile([P, 2 * C], BF16, name="wp_sb")
            mk_sb = const.tile([P, P], BF16, name="mk_sb")
            ones_sb = const.tile([1, P], F32, name="ones_sb")
            qT_sb = const.tile([P, 2 * T], BF16, name="qT_sb")
            kT_sb = const.tile([P, 2 * T], BF16, name="kT_sb")
            va_sb = const.tile([P, NQT * HL * VA], BF16, name="va_sb")
            yT_sb = const.tile([P, 2 * T], BF16, name="yT_sb")

            # ---- input DMAs: everything is a straight [128, X] image copy.
            # wq + the first 512-col xT pieces first so QKV starts early.
            nc.sync.dma_start(out=wq_sb[:, :], in_=wqT[:, :])
            for n in range(KC):
                nc.sync.dma_start(
                    out=xT_sb[:, n * T: n * T + 512],
                    in_=xT[:, n * T: n * T + 512],
                )
            nc.sync.dma_start(out=wk_sb[:, :], in_=wkT[:, :])
            nc.sync.dma_start(out=wv_sb[:, :], in_=wvT[:, :])
            for n in range(KC):
                nc.sync.dma_start(
                    out=xT_sb[:, n * T + 512: (n + 1) * T],
                    in_=xT[:, n * T + 512: (n + 1) * T],
                )
            nc.sync.dma_start(out=wp_sb[:, :], in_=wpT[:, :])
            nc.sync.dma_start(out=mk_sb[:, :], in_=mk[:, :])
            nc.vector.memset(ones_sb[:, :], 1.0)
            va_view = va_sb[:, :].rearrange("p (t h e) -> p t h e", t=NQT, h=HL)
            nc.vector.memset(va_view[:, :, :, D:VA], 1.0)

            # ---- phase 1: QKV projections ----
            # emission order brings heads 0/1 (oc=0) + early v tiles up first
            # so attention can overlap the rest of the phase.
            with tc.tile_pool(name="qkv_ps", bufs=4, space="PSUM") as qkv_pool:
                def qk_tile(w_sb, dst_sb, oc, tch):
                    ps = qkv_pool.tile([P, 512], F32, name="qkps", tag="qkvps")
                    for kc in range(KC):
                        nc.tensor.matmul(
                            ps[:, :],
                            w_sb[:, kc * OL + oc * P: kc * OL + oc * P + P],
                            xT_sb[:, kc * T + tch * 512: kc * T + tch * 512 + 512],
                            start=(kc == 0),
                            stop=(kc == KC - 1),
                        )
                    nc.scalar.copy(
                        dst_sb[:, oc * T + tch * 512: oc * T + tch * 512 + 512],
                        ps[:, :],
                    )

                def v_tile(tt):
                    ps = qkv_pool.tile([P, 512], F32, name="vps", tag="qkvps")
                    for kc in range(KC):
                        nc.tensor.matmul(
                            ps[:, 0:OL],
                            xT_sb[:, kc * T + tt * P: kc * T + tt * P + P],
                            wv_sb[:, kc * OL:(kc + 1) * OL],
                            start=(kc == 0),
                            stop=(kc == KC - 1),
                        )
                    nc.scalar.copy(
                        va_view[:, tt, :, 0:D],
                        ps[:, 0:OL].rearrange("p (h d) -> p h d", h=HL),
                    )

                for tch in range(NQC):
                    qk_tile(wq_sb, qT_sb, 0, tch)
                    qk_tile(wk_sb, kT_sb, 0, tch)
                    for tt in range(4 * tch, 4 * tch + 4):
                        v_tile(tt)
                for tch in range(NQC):
                    qk_tile(wq_sb, qT_sb, 1, tch)
                    qk_tile(wk_sb, kT_sb, 1, tch)

            # ---- phase 2: attention (+ interleaved projection) ----
            # PSUM: s 4 banks, y 2, pr 2.  The softmax-denominator broadcast
            # goes through a DRAM round-trip DMA (stride-0 partition source),
            # keeping PE/ACT/DVE out of that chain entirely.
            with tc.tile_pool(name="s_ps", bufs=3, space="PSUM") as s_pool, \
                 tc.tile_pool(name="y_ps", bufs=2, space="PSUM") as y_pool, \
                 tc.tile_pool(name="dram_sc", bufs=1, space="DRAM") as dram_pool:
                rc_dram = dram_pool.tile([NQC * HL, 512], F32, name="rc_dram")

                def proj_tile(tile_idx):
                    tt, ocn = divmod(tile_idx, 2)
                    trow = tt * P
                    pr_ps = y_pool.tile([P, 512], F32, name="prps", tag="yps")
                    for cc in range(2):
                        nc.tensor.matmul(
                            pr_ps[:, :],
                            yT_sb[:, cc * T + trow: cc * T + trow + P],
                            wp_sb[:, cc * C + ocn * 512: cc * C + ocn * 512 + 512],
                            start=(cc == 0),
                            stop=(cc == 1),
                        )
                    o_sb = outp.tile([P, 512], F32, name="osb", tag="osb")
                    # alternate eviction engine: a single engine's in-order
                    # queue serializes the slot releases of the 2-slot pool
                    if tile_idx % 2 == 0:
                        nc.scalar.copy(o_sb[:, :], pr_ps[:, :])
                    else:
                        nc.vector.tensor_copy(o_sb[:, :], pr_ps[:, :])
                    nc.sync.dma_start(
                        out=out[trow:trow + P, ocn * 512:(ocn + 1) * 512],
                        in_=o_sb[:, :],
                    )

                for j4 in range(NQC):
                    q0 = j4 * 512
                    for hp in range(2):
                        # two heads interleaved per k-tile: one shared 2-bank
                        # S tile, one wide exp for both heads (the +352cyc
                        # ACTIVATE pipeline fill amortizes over 1024 cols),
                        # two independent y accumulations.  Doubles the
                        # PE-side work available per ACT op.
                        h0, h1 = 2 * hp, 2 * hp + 1
                        ch = hp
                        y0 = y_pool.tile([P, 512], F32, name="yps0", tag="yps")
                        y1 = y_pool.tile([P, 512], F32, name="yps1", tag="yps")
                        nk = 4 * (j4 + 1)
                        for i in range(nk):
                            m0 = max(0, i - 4 * j4)
                            c0 = P * m0
                            s2 = s_pool.tile([P, 1024], F32, name="sps", tag="sps")
                            for half, po in ((0, 0), (1, 64)):
                                nc.tensor.matmul(
                                    s2[:, half * 512 + c0: half * 512 + 512],
                                    kT_sb[po:po + D, ch * T + i * P: ch * T + i * P + P],
                                    qT_sb[po:po + D, ch * T + q0 + c0: ch * T + q0 + 512],
                                    start=True,
                                    stop=True,
                                )
                            p2 = att.tile([P, 1024], BF16, name="pt", tag="pt")
                            if m0 == 0:
                                nc.scalar.activation(
                                    p2[:, 0:1024], s2[:, 0:1024], Exp, scale=0.125
                                )
                            else:
                                # diagonal: the two live spans are disjoint,
                                # exp each half so no unwritten PSUM is read
                                for half in range(2):
                                    nc.scalar.activation(
                                        p2[:, half * 512 + c0: half * 512 + 512],
                                        s2[:, half * 512 + c0: half * 512 + 512],
                                        Exp, scale=0.125,
                                    )
                            if i >= 4 * j4:
                                for half in range(2):
                                    nc.gpsimd.tensor_mul(
                                        p2[:, half * 512 + c0: half * 512 + c0 + P],
                                        p2[:, half * 512 + c0: half * 512 + c0 + P],
                                        mk_sb[:, :],
                                    )
                            for half, y_ps, hh in ((0, y0, h0), (1, y1, h1)):
                                nc.tensor.matmul(
                                    y_ps[0:VA, c0:512],
                                    va_sb[:, (i * HL + hh) * VA:(i * HL + hh) * VA + VA],
                                    p2[:, half * 512 + c0: half * 512 + 512],
                                    start=(i == 0),
                                    stop=(i == nk - 1),
                                )

                        tails = [(0, y0, h0), (1, y1, h1)]
                        if j4 == NQC - 1 and hp == 1:
                            tails.reverse()  # gating head's short chain first
                        for half, y_ps, hh in tails:
                            po = 64 * half
                            bc_sb = misc.tile([P, 512], F32, name="bcsb", tag="bcsb")
                            last = (j4 == NQC - 1 and hh == HL - 1)
                            if not last:
                                # evict y to SBUF (frees the PSUM bank), then
                                # normalize: reciprocal row -> DRAM ->
                                # broadcast DMA across 128 partitions -> one
                                # DVE multiply.  All off the PE critical path.
                                y_sb = misc.tile([P, 512], F32, name="ysb", tag="ysb")
                                nc.vector.tensor_copy(y_sb[0:VA, :], y_ps[0:VA, :])
                                rc = misc.tile([1, 512], F32, name="rc", tag="rc")
                                nc.vector.reciprocal(rc[:, :], y_sb[D:VA, :])
                                slot = j4 * HL + hh
                                nc.sync.dma_start(
                                    out=rc_dram[slot:slot + 1, :], in_=rc[:, :]
                                )
                                nc.sync.dma_start(
                                    out=bc_sb[:, :],
                                    in_=rc_dram[slot:slot + 1, :].to_broadcast((P, 512)),
                                )
                                nc.vector.tensor_mul(
                                    yT_sb[po:po + D, ch * T + q0: ch * T + q0 + 512],
                                    y_sb[0:D, :],
                                    bc_sb[0:D, :],
                                )
                            else:
                                # the very last head gates the final
                                # projection burst: shortest-latency chain
                                rc = misc.tile([1, 512], F32, name="rc", tag="rc")
                                nc.vector.reciprocal(rc[:, :], y_ps[D:VA, :])
                                bc_ps = y_pool.tile([P, 512], F32, name="prps", tag="yps")
                                nc.tensor.matmul(
                                    bc_ps[:, :], ones_sb[0:1, :], rc[:, :],
                                    start=True, stop=True,
                                )
                                bc2 = misc.tile([P, 512], F32, name="bcsb", tag="bcsb")
                                nc.vector.tensor_copy(bc2[:, :], bc_ps[:, :])
                                nc.vector.tensor_mul(
                                    yT_sb[po:po + D, ch * T + q0: ch * T + q0 + 512],
                                    y_ps[0:D, :],
                                    bc2[0:D, :],
                                )

                        # previous chunk's projection after the evictions
                        # (proj tiles share the y pool's PSUM slots)
                        if j4 > 0:
                            for k in range(4):
                                proj_tile((j4 - 1) * 8 + hp * 4 + k)
                    if j4 == NQC - 1:
                        for k in range(8):
                            proj_tile(j4 * 8 + k)
    nc.compile()
    return nc


def _get_nc():
    global _nc_cache
    if _nc_cache is None:
        _nc_cache = _build_nc()
    return _nc_cache


def _prepare_in_maps(x, W_qkv, W_proj):
    x = np.asarray(x, np.float32)
    W_qkv = np.asarray(W_qkv, np.float32)
    W_proj = np.asarray(W_proj, np.float32)
    # [r, j] = 1 where j >= r (upper triangle incl diag, in S^T [k, q] layout)
    tri = (np.arange(P)[None, :] >= np.arange(P)[:, None]).astype(np.float32)
    tri = tri.astype(_BF16)
    in_maps = []
    for c in range(8):
        b, g = c // 4, c % 4
        r0 = OL * g
        def img(a):
            # [R, Y] with R = n*128 rows -> SBUF image [128, n*Y]
            n = a.shape[0] // P
            return np.ascontiguousarray(
                a.reshape(n, P, a.shape[1]).transpose(1, 0, 2).reshape(P, -1)
            ).astype(_BF16)

        in_maps.append({
            "xT": img(x[b].T),
            "wqT": img(W_qkv[r0:r0 + OL, :].T),
            "wkT": img(W_qkv[C + r0:C + r0 + OL, :].T),
            "wvT": img(W_qkv[2 * C + r0:2 * C + r0 + OL, :].T),
            "wpT": img(W_proj[:, r0:r0 + OL].T),
            "mask_tri": tri,
        })
    return in_maps


def _combine(results):
    out = np.zeros((B, T, C), np.float32)
    for c in range(8):
        out[c // 4] += results[c]["out"]
    return out


def kernel(x, W_qkv, W_proj):
    nc = _get_nc()
    in_maps = _prepare_in_maps(x, W_qkv, W_proj)
    try:
        res = bass_utils.run_bass_kernel_spmd(nc, in_maps, core_ids=list(range(8)))
    except Exception:
        # rare transient NRT device errors; one retry
        res = bass_utils.run_bass_kernel_spmd(nc, in_maps, core_ids=list(range(8)))
    return _combine(res.results)


def kernel_traced(x, W_qkv, W_proj, trace_cores=None):
    """Like kernel() but returns (out, exec_time_ns) using an NTFF profile."""
    nc = _get_nc()
    in_maps = _prepare_in_maps(x, W_qkv, W_proj)
    res = bass_utils.run_bass_kernel_spmd(
        nc, in_maps, core_ids=list(range(8)), trace=True, trace_cores=trace_cores
    )
    return _combine(res.results), res.exec_time_ns



# revision 8
# speedup vs baseline: 1.6828x; 1.6828x over previous
"""Trainium2 Bass kernel: causal self-attention (B=2, T=2048, C=1024, H=16, Dh=64).

Sharding: 8 cores = 2 (batch) x 4 (head groups of 4 heads).  Each core gets
x[b] plus the W_qkv rows / W_proj columns for its heads, computes the full
attention + a partial output projection for its batch, and the host sums the
4 partials per batch (tensor-parallel unshard).

All matmuls run in bf16 with f32 PSUM accumulation.  x is passed transposed
(xT = x[b].T) so that:
  qT, kT = Wq @ xT, Wk @ xT     (head dim on partitions)  -- no transposes
  v      = xT.T @ WvT           (natural [T, d] layout)
  S^T    = kT_h(tile).T @ qT_h  ([k, q] layout, 128x512 blocks)
  exp on ScalarE (logits are bounded, no max pass needed); causal masking by
  computing only the live columns of each block (diagonal blocks truncate
  their dead leading columns in the S matmul, the exp, and the y matmul) plus
  one multiplicative [128,128] triangle mask on the diagonal subtile; row
  sums via a ones column appended to V (so P@[V|1] accumulates y^T and the
  softmax denominators in one PSUM tile); the 1/sum normalization applied on
  eviction using a PE-broadcast reciprocal row.
  out_partial = y^T.T @ WpT   (f32, DMA'd out).

ScalarE runs nothing but Exp during the attention phase -- any other
activation function (even Copy) can trigger a ~2.7us ACT table reload.
"""
import sys
import types

import numpy as np
import ml_dtypes

_BF16 = ml_dtypes.bfloat16


def _install_ntff_hook():
    """Provide antenv.axon_hooks so run_bass_kernel_spmd(trace=True) works."""
    if "antenv.axon_hooks" in sys.modules:
        return
    mod = types.ModuleType("antenv.axon_hooks")
    mod._hook = None

    def set_axon_ntff_profile_hook(h):
        mod._hook = h

    def get_axon_ntff_profile_hook():
        return mod._hook

    mod.set_axon_ntff_profile_hook = set_axon_ntff_profile_hook
    mod.get_axon_ntff_profile_hook = get_axon_ntff_profile_hook
    sys.modules["antenv.axon_hooks"] = mod
    try:
        import antenv

        antenv.axon_hooks = mod
    except Exception:
        pass
    try:
        from trn_agent_boot.trn_boot import _ntff_profile_via_ctypes

        mod.set_axon_ntff_profile_hook(
            _ntff_profile_via_ctypes("/opt/axon/libaxon_pjrt.so")
        )
    except Exception:
        pass


_install_ntff_hook()

import concourse.bacc as bacc
import concourse.mybir as mybir
from concourse import bass_utils
from concourse.tile import TileContext

# no network bucket in this container; keep artifacts local
bass_utils.upload_artifacts = lambda tmpdir: tmpdir

BF16 = mybir.dt.bfloat16
F32 = mybir.dt.float32

B, T, C = 2, 2048, 1024
H, D = 16, 64
HL = 4            # heads per core
OL = HL * D       # 256 local qkv output dim
P = 128
KC = C // P       # 8 contraction chunks
NQT = T // P      # 16 q/k 128-tiles
NQC = T // 512    # 4 q 512-chunks
VA = D + 1        # v columns per head incl. ones column (65)

_nc_cache = None


def _build_nc():
    nc = bacc.Bacc("TRN2", target_bir_lowering=False, debug=False, num_devices=8)

    # all inputs arrive pre-arranged in SBUF-image layout [128, X] so every
    # input DMA moves multi-KB contiguous runs per partition row.
    xT = nc.declare_dram_parameter("xT", [P, KC * T], BF16, isOutput=False)
    wqT = nc.declare_dram_parameter("wqT", [P, KC * OL], BF16, isOutput=False)
    wkT = nc.declare_dram_parameter("wkT", [P, KC * OL], BF16, isOutput=False)
    wvT = nc.declare_dram_parameter("wvT", [P, KC * OL], BF16, isOutput=False)
    wpT = nc.declare_dram_parameter("wpT", [P, 2 * C], BF16, isOutput=False)
    mk = nc.declare_dram_parameter("mask_tri", [P, P], BF16, isOutput=False)
    out = nc.declare_dram_parameter("out", [T, C], F32, isOutput=True)

    Exp = mybir.ActivationFunctionType.Exp

    with TileContext(nc) as tc:
        with tc.tile_pool(name="const", bufs=1) as const, \
             tc.tile_pool(name="misc", bufs=3) as misc, \
             tc.tile_pool(name="att", bufs=8) as att, \
             tc.tile_pool(name="outp", bufs=6) as outp:
            xT_sb = const.tile([P, KC * T], BF16, name="xT_sb")
            wq_sb = const.tile([P, KC * OL], BF16, name="wq_sb")
            wk_sb = const.tile([P, KC * OL], BF16, name="wk_sb")
            wv_sb = const.tile([P, KC * OL], BF16, name="wv_sb")
            wp_sb = const.tile([P, 2 * C], BF16, name="wp_sb")
            mk_sb = const.tile([P, P], BF16, name="mk_sb")
            ones_sb = const.tile([1, P], F32, name="ones_sb")
            qT_sb = const.tile([P, 2 * T], BF16, name="qT_sb")
            kT_sb = const.tile([P, 2 * T], BF16, name="kT_sb")
            va_sb = const.tile([P, NQT * HL * VA], BF16, name="va_sb")
            yT_sb = const.tile([P, 2 * T], BF16, name="yT_sb")

            # ---- input DMAs: everything is a straight [128, X] image copy.
            # wq per-kc pieces + the first 512-col xT pieces first so the
            # kc-loop of the first QKV tile can start as soon as possible.
            for n in range(KC):
                nc.sync.dma_start(
                    out=wq_sb[:, n * OL:(n + 1) * OL],
                    in_=wqT[:, n * OL:(n + 1) * OL],
                )
                nc.sync.dma_start(
                    out=xT_sb[:, n * T: n * T + 512],
                    in_=xT[:, n * T: n * T + 512],
                )
            nc.sync.dma_start(out=wk_sb[:, :], in_=wkT[:, :])
            nc.sync.dma_start(out=wv_sb[:, :], in_=wvT[:, :])
            for n in range(KC):
                nc.sync.dma_start(
                    out=xT_sb[:, n * T + 512: (n + 1) * T],
                    in_=xT[:, n * T + 512: (n + 1) * T],
                )
            nc.sync.dma_start(out=wp_sb[:, :], in_=wpT[:, :])
            nc.sync.dma_start(out=mk_sb[:, :], in_=mk[:, :])
            nc.vector.memset(ones_sb[:, :], 1.0)
            va_view = va_sb[:, :].rearrange("p (t h e) -> p t h e", t=NQT, h=HL)
            nc.vector.memset(va_view[:, :, :, D:VA], 1.0)

            # ---- phase 1: QKV projections ----
            # emission order brings heads 0/1 (oc=0) + early v tiles up first
            # so attention can overlap the rest of the phase.
            with tc.tile_pool(name="qkv_ps", bufs=4, space="PSUM") as qkv_pool:
                def qk_tile(w_sb, dst_sb, oc, tch):
                    ps = qkv_pool.tile([P, 512], F32, name="qkps", tag="qkvps")
                    for kc in range(KC):
                        nc.tensor.matmul(
                            ps[:, :],
                            w_sb[:, kc * OL + oc * P: kc * OL + oc * P + P],
                            xT_sb[:, kc * T + tch * 512: kc * T + tch * 512 + 512],
                            start=(kc == 0),
                            stop=(kc == KC - 1),
                        )
                    nc.scalar.copy(
                        dst_sb[:, oc * T + tch * 512: oc * T + tch * 512 + 512],
                        ps[:, :],
                    )

                def v_tile(tt):
                    ps = qkv_pool.tile([P, 512], F32, name="vps", tag="qkvps")
                    for kc in range(KC):
                        nc.tensor.matmul(
                            ps[:, 0:OL],
                            xT_sb[:, kc * T + tt * P: kc * T + tt * P + P],
                            wv_sb[:, kc * OL:(kc + 1) * OL],
                            start=(kc == 0),
                            stop=(kc == KC - 1),
                        )
                    nc.scalar.copy(
                        va_view[:, tt, :, 0:D],
                        ps[:, 0:OL].rearrange("p (h d) -> p h d", h=HL),
                    )

                for tch in range(NQC):
                    qk_tile(wq_sb, qT_sb, 0, tch)
                    qk_tile(wk_sb, kT_sb, 0, tch)
                    for tt in range(4 * tch, 4 * tch + 4):
                        v_tile(tt)
                for tch in range(NQC):
                    qk_tile(wq_sb, qT_sb, 1, tch)
                    qk_tile(wk_sb, kT_sb, 1, tch)

            # ---- phase 2: attention (+ interleaved projection) ----
            # PSUM: s 4 banks, y 2, pr 2.  The softmax-denominator broadcast
            # goes through a DRAM round-trip DMA (stride-0 partition source),
            # keeping PE/ACT/DVE out of that chain entirely.
            with tc.tile_pool(name="s_ps", bufs=3, space="PSUM") as s_pool, \
                 tc.tile_pool(name="y_ps", bufs=2, space="PSUM") as y_pool, \
                 tc.tile_pool(name="dram_sc", bufs=1, space="DRAM") as dram_pool:
                rc_dram = dram_pool.tile([NQC * HL, 512], F32, name="rc_dram")

                def proj_tile(tile_idx):
                    tt, ocn = divmod(tile_idx, 2)
                    trow = tt * P
                    pr_ps = y_pool.tile([P, 512], F32, name="prps", tag="yps")
                    for cc in range(2):
                        nc.tensor.matmul(
                            pr_ps[:, :],
                            yT_sb[:, cc * T + trow: cc * T + trow + P],
                            wp_sb[:, cc * C + ocn * 512: cc * C + ocn * 512 + 512],
                            start=(cc == 0),
                            stop=(cc == 1),
                        )
                    o_sb = outp.tile([P, 512], F32, name="osb", tag="osb")
                    # alternate eviction engine: a single engine's in-order
                    # queue serializes the slot releases of the 2-slot pool
                    if tile_idx % 2 == 0:
                        nc.scalar.copy(o_sb[:, :], pr_ps[:, :])
                    else:
                        nc.vector.tensor_copy(o_sb[:, :], pr_ps[:, :])
                    nc.sync.dma_start(
                        out=out[trow:trow + P, ocn * 512:(ocn + 1) * 512],
                        in_=o_sb[:, :],
                    )

                for j4 in range(NQC):
                    q0 = j4 * 512
                    for hp in range(2):
                        # two heads interleaved per k-tile: one shared 2-bank
                        # S tile, one wide exp for both heads (the +352cyc
                        # ACTIVATE pipeline fill amortizes over 1024 cols),
                        # two independent y accumulations.  Doubles the
                        # PE-side work available per ACT op.
                        h0, h1 = 2 * hp, 2 * hp + 1
                        ch = hp
                        y0 = y_pool.tile([P, 512], F32, name="yps0", tag="yps")
                        y1 = y_pool.tile([P, 512], F32, name="yps1", tag="yps")
                        nk = 4 * (j4 + 1)
                        # diagonal k-tiles first: their exp->mask->PV chains
                        # overlap the full tiles, so the last PV (stop=True)
                        # has no gpsimd mask on its critical path.
                        iorder = list(range(4 * j4, nk)) + list(range(0, 4 * j4))
                        for iidx, i in enumerate(iorder):
                            m0 = max(0, i - 4 * j4)
                            c0 = P * m0
                            s2 = s_pool.tile([P, 1024], F32, name="sps", tag="sps")
                            for half, po in ((0, 0), (1, 64)):
                                nc.tensor.matmul(
                                    s2[:, half * 512 + c0: half * 512 + 512],
                                    kT_sb[po:po + D, ch * T + i * P: ch * T + i * P + P],
                                    qT_sb[po:po + D, ch * T + q0 + c0: ch * T + q0 + 512],
                                    start=True,
                                    stop=True,
                                )
                            p2 = att.tile([P, 1024], BF16, name="pt", tag="pt")
                            if m0 == 0:
                                nc.scalar.activation(
                                    p2[:, 0:1024], s2[:, 0:1024], Exp, scale=0.125
                                )
                            else:
                                # diagonal: the two live spans are disjoint,
                                # exp each half so no unwritten PSUM is read
                                for half in range(2):
                                    nc.scalar.activation(
                                        p2[:, half * 512 + c0: half * 512 + 512],
                                        s2[:, half * 512 + c0: half * 512 + 512],
                                        Exp, scale=0.125,
                                    )
                            if i >= 4 * j4:
                                for half in range(2):
                                    nc.gpsimd.tensor_mul(
                                        p2[:, half * 512 + c0: half * 512 + c0 + P],
                                        p2[:, half * 512 + c0: half * 512 + c0 + P],
                                        mk_sb[:, :],
                                    )
                            for half, y_ps, hh in ((0, y0, h0), (1, y1, h1)):
                                nc.tensor.matmul(
                                    y_ps[0:VA, c0:512],
                                    va_sb[:, (i * HL + hh) * VA:(i * HL + hh) * VA + VA],
                                    p2[:, half * 512 + c0: half * 512 + 512],
                                    start=(iidx == 0),
                                    stop=(iidx == nk - 1),
                                )

                        tails = [(0, y0, h0), (1, y1, h1)]
                        if j4 == NQC - 1 and hp == 1:
                            tails.reverse()  # gating head's short chain first
                        for half, y_ps, hh in tails:
                            po = 64 * half
                            bc_sb = misc.tile([P, 512], F32, name="bcsb", tag="bcsb")
                            last = (j4 == NQC - 1 and hh == HL - 1)
                            if not last:
                                # evict y to SBUF (frees the PSUM bank), then
                                # normalize: reciprocal row -> DRAM ->
                                # broadcast DMA across 128 partitions -> one
                                # DVE multiply.  All off the PE critical path.
                                y_sb = misc.tile([P, 512], F32, name="ysb", tag="ysb")
                                nc.vector.tensor_copy(y_sb[0:D, :], y_ps[0:D, :])
                                d_sb = misc.tile([1, 512], F32, name="dsb", tag="dsb")
                                nc.vector.tensor_copy(d_sb[:, :], y_ps[D:VA, :])
                                rc = misc.tile([1, 512], F32, name="rc", tag="rc")
                                nc.vector.reciprocal_approx_fast(rc[:, :], d_sb[:, :])
                                slot = j4 * HL + hh
                                nc.sync.dma_start(
                                    out=rc_dram[slot:slot + 1, :], in_=rc[:, :]
                                )
                                nc.sync.dma_start(
                                    out=bc_sb[:, :],
                                    in_=rc_dram[slot:slot + 1, :].to_broadcast((P, 512)),
                                )
                                nc.vector.tensor_mul(
                                    yT_sb[po:po + D, ch * T + q0: ch * T + q0 + 512],
                                    y_sb[0:D, :],
                                    bc_sb[0:D, :],
                                )
                            else:
                                # the very last head gates the final
                                # projection burst: shortest-latency chain
                                d_sb = misc.tile([1, 512], F32, name="dsb", tag="dsb")
                                nc.vector.tensor_copy(d_sb[:, :], y_ps[D:VA, :])
                                rc = misc.tile([1, 512], F32, name="rc", tag="rc")
                                nc.vector.reciprocal_approx_fast(rc[:, :], d_sb[:, :])
                                bc_ps = y_pool.tile([P, 512], F32, name="prps", tag="yps")
                                nc.tensor.matmul(
                                    bc_ps[:, :], ones_sb[0:1, :], rc[:, :],
                                    start=True, stop=True,
                                )
                                bc2 = misc.tile([P, 512], F32, name="bcsb", tag="bcsb")
                                nc.vector.tensor_copy(bc2[:, :], bc_ps[:, :])
                                nc.vector.tensor_mul(
                                    yT_sb[po:po + D, ch * T + q0: ch * T + q0 + 512],
                                    y_ps[0:D, :],
                                    bc2[0:D, :],
                                )

                        # previous chunk's projection after the evictions
                        # (proj tiles share the y pool's PSUM slots)
                        if j4 > 0:
                            for k in range(4):
                                proj_tile((j4 - 1) * 8 + hp * 4 + k)
                    if j4 == NQC - 1:
                        for k in range(8):
                            proj_tile(j4 * 8 + k)
    nc.compile()
    return nc


def _get_nc():
    global _nc_cache
    if _nc_cache is None:
        _nc_cache = _build_nc()
    return _nc_cache


def _prepare_in_maps(x, W_qkv, W_proj):
    x = np.asarray(x, np.float32)
    W_qkv = np.asarray(W_qkv, np.float32)
    W_proj = np.asarray(W_proj, np.float32)
    # [r, j] = 1 where j >= r (upper triangle incl diag, in S^T [k, q] layout)
    tri = (np.arange(P)[None, :] >= np.arange(P)[:, None]).astype(np.float32)
    tri = tri.astype(_BF16)
    in_maps = []
    for c in range(8):
        b, g = c // 4, c % 4
        r0 = OL * g
        def img(a):
            # [R, Y] with R = n*128 rows -> SBUF image [128, n*Y]
            n = a.shape[0] // P
            return np.ascontiguousarray(
                a.reshape(n, P, a.shape[1]).transpose(1, 0, 2).reshape(P, -1)
            ).astype(_BF16)

        in_maps.append({
            "xT": img(x[b].T),
            "wqT": img(W_qkv[r0:r0 + OL, :].T),
            "wkT": img(W_qkv[C + r0:C + r0 + OL, :].T),
            "wvT": img(W_qkv[2 * C + r0:2 * C + r0 + OL, :].T),
            "wpT": img(W_proj[:, r0:r0 + OL].T),
            "mask_tri": tri,
        })
    return in_maps


def _combine(results):
    out = np.zeros((B, T, C), np.float32)
    for c in range(8):
        out[c // 4] += results[c]["out"]
    return out


def kernel(x, W_qkv, W_proj):
    nc = _get_nc()
    in_maps = _prepare_in_maps(x, W_qkv, W_proj)
    try:
        res = bass_utils.run_bass_kernel_spmd(nc, in_maps, core_ids=list(range(8)))
    except Exception:
        # rare transient NRT device errors; one retry
        res = bass_utils.run_bass_kernel_spmd(nc, in_maps, core_ids=list(range(8)))
    return _combine(res.results)


def kernel_traced(x, W_qkv, W_proj, trace_cores=None):
    """Like kernel() but returns (out, exec_time_ns) using an NTFF profile."""
    nc = _get_nc()
    in_maps = _prepare_in_maps(x, W_qkv, W_proj)
    res = bass_utils.run_bass_kernel_spmd(
        nc, in_maps, core_ids=list(range(8)), trace=True, trace_cores=trace_cores
    )
    return _combine(res.results), res.exec_time_ns



# revision 9
# speedup vs baseline: 1.6849x; 1.0012x over previous
"""Trainium2 Bass kernel v2: causal self-attention, fused-phase emission.

Sharding: 8 cores = 2 (batch) x 4 (head groups of 4 heads).  Each core gets
x[b] plus the W_qkv rows / W_proj columns for its heads, computes the full
attention + a partial output projection for its batch, and the host sums the
4 partials per batch (tensor-parallel unshard).

v2 structure: the QKV projection, attention, and output projection are
emitted as ONE interleaved stream per 512-query chunk so the ScalarE exp
stream (the secondary bottleneck, ~1 col/cycle, ~87us total) starts ~8us in
and runs concurrently with the PE work (~113us total):

  upfront: qkv tiles for chunk 0
  chunk t: attention (2 head-pairs), with qkv tiles of chunk t+1 and
           projection tiles of chunk t-1 woven between PV steps as PE
           filler (the PE queue is strictly in-order; filler keeps it
           dense while ACT works through the exp queue).

PSUM (8 banks): qkv/proj pool 2x[128,512] (2), S pool 2x[128,1024] (4),
y pool 2x[128,512] (2).

ScalarE runs ONLY Exp (one warmup exp at t~0 prefetches the ACT table
during the input-DMA head); all PSUM evictions go to DVE/GpSimd.
"""
import sys
import types

import numpy as np
import ml_dtypes

_BF16 = ml_dtypes.bfloat16


def _install_ntff_hook():
    """Provide antenv.axon_hooks so run_bass_kernel_spmd(trace=True) works."""
    if "antenv.axon_hooks" in sys.modules:
        return
    mod = types.ModuleType("antenv.axon_hooks")
    mod._hook = None

    def set_axon_ntff_profile_hook(h):
        mod._hook = h

    def get_axon_ntff_profile_hook():
        return mod._hook

    mod.set_axon_ntff_profile_hook = set_axon_ntff_profile_hook
    mod.get_axon_ntff_profile_hook = get_axon_ntff_profile_hook
    sys.modules["antenv.axon_hooks"] = mod
    try:
        import antenv

        antenv.axon_hooks = mod
    except Exception:
        pass
    try:
        from trn_agent_boot.trn_boot import _ntff_profile_via_ctypes

        mod.set_axon_ntff_profile_hook(
            _ntff_profile_via_ctypes("/opt/axon/libaxon_pjrt.so")
        )
    except Exception:
        pass


_install_ntff_hook()

import concourse.bacc as bacc
import concourse.mybir as mybir
from concourse import bass_utils
from concourse.tile import TileContext

# no network bucket in this container; keep artifacts local
bass_utils.upload_artifacts = lambda tmpdir: tmpdir

BF16 = mybir.dt.bfloat16
F32 = mybir.dt.float32

B, T, C = 2, 2048, 1024
H, D = 16, 64
HL = 4            # heads per core
OL = HL * D       # 256 local qkv output dim
P = 128
KC = C // P       # 8 contraction chunks
NQT = T // P      # 16 q/k 128-tiles
NQC = T // 512    # 4 q 512-chunks
VA = D + 1        # v columns per head incl. ones column (65)

_nc_cache = None


def _build_nc():
    nc = bacc.Bacc("TRN2", target_bir_lowering=False, debug=False, num_devices=8)

    # all inputs arrive pre-arranged in SBUF-image layout [128, X] so every
    # input DMA moves multi-KB contiguous runs per partition row.
    xT = nc.declare_dram_parameter("xT", [P, KC * T], BF16, isOutput=False)
    wqT = nc.declare_dram_parameter("wqT", [P, KC * OL], BF16, isOutput=False)
    wkT = nc.declare_dram_parameter("wkT", [P, KC * OL], BF16, isOutput=False)
    wvT = nc.declare_dram_parameter("wvT", [P, KC * OL], BF16, isOutput=False)
    wpT = nc.declare_dram_parameter("wpT", [P, 2 * C], BF16, isOutput=False)
    mk = nc.declare_dram_parameter("mask_tri", [P, P], BF16, isOutput=False)
    out = nc.declare_dram_parameter("out", [T, C], BF16, isOutput=True)
    out_b = nc.declare_dram_parameter("out_b", [512, C], BF16, isOutput=True)

    Exp = mybir.ActivationFunctionType.Exp

    with TileContext(nc) as tc:
        with tc.tile_pool(name="const", bufs=1) as const, \
             tc.tile_pool(name="misc", bufs=3) as misc, \
             tc.tile_pool(name="att", bufs=8) as att, \
             tc.tile_pool(name="outp", bufs=6) as outp, \
             tc.tile_pool(name="mm_ps", bufs=2, space="PSUM") as mm_pool, \
             tc.tile_pool(name="s_ps", bufs=2, space="PSUM") as s_pool, \
             tc.tile_pool(name="y_ps", bufs=2, space="PSUM") as y_pool, \
             tc.tile_pool(name="dram_sc", bufs=1, space="DRAM") as dram_pool:
            xT_sb = const.tile([P, KC * T], BF16, name="xT_sb")
            wq_sb = const.tile([P, KC * OL], BF16, name="wq_sb")
            wk_sb = const.tile([P, KC * OL], BF16, name="wk_sb")
            wv_sb = const.tile([P, KC * OL], BF16, name="wv_sb")
            wp_sb = const.tile([P, 2 * C], BF16, name="wp_sb")
            mk_sb = const.tile([P, P], BF16, name="mk_sb")
            ones_sb = const.tile([1, P], F32, name="ones_sb")
            ones_bf = const.tile([1, P], BF16, name="ones_bf")
            qT_sb = const.tile([P, 2 * T], BF16, name="qT_sb")
            kT_sb = const.tile([P, 2 * T], BF16, name="kT_sb")
            va_sb = const.tile([P, NQT * HL * VA], BF16, name="va_sb")
            yT_sb = const.tile([P, 2 * T], BF16, name="yT_sb")
            warm_sb = const.tile([1, 16], F32, name="warm_sb")
            rc_dram = dram_pool.tile([NQC * HL, 512], F32, name="rc_dram")

            va_view = va_sb[:, :].rearrange("p (t h e) -> p t h e", t=NQT, h=HL)

            # ---- input DMAs, ordered by first use.  The xT image is
            # grouped by 512-query chunk on the host, so each chunk is ONE
            # contiguous ~1MB transfer (split in halves for earlier start).
            GW = KC * 512  # columns per q-chunk group
            nc.sync.dma_start(out=wq_sb[:, 0:OL], in_=wqT[:, 0:OL])
            nc.sync.dma_start(out=xT_sb[:, 0:GW // 2], in_=xT[:, 0:GW // 2])
            nc.sync.dma_start(out=wq_sb[:, OL:], in_=wqT[:, OL:])
            nc.sync.dma_start(out=xT_sb[:, GW // 2:GW], in_=xT[:, GW // 2:GW])
            nc.sync.dma_start(out=wk_sb[:, :], in_=wkT[:, :])
            nc.sync.dma_start(out=wv_sb[:, :], in_=wvT[:, :])
            nc.sync.dma_start(out=mk_sb[:, :], in_=mk[:, :])
            nc.sync.dma_start(out=xT_sb[:, GW:2 * GW], in_=xT[:, GW:2 * GW])
            nc.sync.dma_start(out=wp_sb[:, :], in_=wpT[:, :])
            nc.sync.dma_start(out=xT_sb[:, 2 * GW:3 * GW], in_=xT[:, 2 * GW:3 * GW])
            nc.sync.dma_start(out=xT_sb[:, 3 * GW:], in_=xT[:, 3 * GW:])

            nc.vector.memset(ones_sb[:, :], 1.0)
            nc.vector.memset(ones_bf[:, :], 1.0)
            nc.vector.memset(warm_sb[:, :], 0.0)
            nc.vector.memset(va_view[:, :, :, D:VA], 1.0)
            # warmup: pulls the exp ACT table load into the DMA head.
            warm_p = misc.tile([1, 16], BF16, name="warm_p", tag="rc")
            nc.scalar.activation(warm_p[:, :], warm_sb[:, :], Exp, scale=1.0)
            # PE prewarm: ~3us of full-array dummy matmuls (no DMA
            # dependency) flip the HAM clock gate to 8/8 before the first
            # real matmul arrives.
            warm_r = const.tile([P, 512], BF16, name="warm_r")
            nc.vector.memset(warm_r[:, :], 0.0)
            warm_ps = y_pool.tile([P, 512], F32, name="warmps", tag="yps")
            for _ in range(14):
                nc.tensor.matmul(
                    warm_ps[:, :], warm_r[:, 0:P], warm_r[:, :],
                    start=True, stop=True,
                )

            # ---- tile builders ----
            def qk_tile(w_sb, dst_sb, oc, tch):
                ps = mm_pool.tile([P, 512], F32, name="qkps", tag="mmps")
                for kc in range(KC):
                    nc.tensor.matmul(
                        ps[:, :],
                        w_sb[:, kc * OL + oc * P: kc * OL + oc * P + P],
                        xT_sb[:, (tch * KC + kc) * 512: (tch * KC + kc) * 512 + 512],
                        start=(kc == 0),
                        stop=(kc == KC - 1),
                    )
                ev = nc.scalar.copy if tch <= 1 else nc.vector.tensor_copy
                ev(
                    dst_sb[:, oc * T + tch * 512: oc * T + tch * 512 + 512],
                    ps[:, :],
                )

            def v_tile(tt):
                ps = mm_pool.tile([P, 512], F32, name="vps", tag="mmps")
                for kc in range(KC):
                    nc.tensor.matmul(
                        ps[:, 0:OL],
                        xT_sb[:, ((tt // 4) * KC + kc) * 512 + (tt % 4) * P:
                              ((tt // 4) * KC + kc) * 512 + (tt % 4) * P + P],
                        wv_sb[:, kc * OL:(kc + 1) * OL],
                        start=(kc == 0),
                        stop=(kc == KC - 1),
                    )
                nc.vector.tensor_copy(
                    va_view[:, tt, :, 0:D],
                    ps[:, 0:OL].rearrange("p (h d) -> p h d", h=HL),
                )

            def proj_tile(tile_idx, ccs=(0, 1), dst=None, drow0=0, evict=None):
                tt, ocn = divmod(tile_idx, 2)
                trow = tt * P
                dst = out if dst is None else dst
                pr_ps = mm_pool.tile([P, 512], F32, name="prps", tag="mmps")
                for cc in ccs:
                    nc.tensor.matmul(
                        pr_ps[:, :],
                        yT_sb[:, cc * T + trow: cc * T + trow + P],
                        wp_sb[:, cc * C + ocn * 512: cc * C + ocn * 512 + 512],
                        start=(cc == ccs[0]),
                        stop=(cc == ccs[-1]),
                    )
                o_sb = outp.tile([P, 512], BF16, name="osb", tag="osb")
                # alternate eviction engine so one engine's in-order queue
                # doesn't serialize the slot releases
                if evict is None:
                    evict = "vector" if tile_idx % 2 == 0 else "scalar"
                if evict == "vector":
                    nc.vector.tensor_copy(o_sb[:, :], pr_ps[:, :])
                else:
                    nc.scalar.copy(o_sb[:, :], pr_ps[:, :])
                nc.sync.dma_start(
                    out=dst[trow - drow0:trow - drow0 + P,
                            ocn * 512:(ocn + 1) * 512],
                    in_=o_sb[:, :],
                )

            # ---- fused per-chunk emission ----
            # upfront: chunk 0's qkv tiles
            qk_tile(wq_sb, qT_sb, 0, 0)
            qk_tile(wk_sb, kT_sb, 0, 0)
            for tt in range(4):
                v_tile(tt)
            qk_tile(wq_sb, qT_sb, 1, 0)
            qk_tile(wk_sb, kT_sb, 1, 0)

            for j4 in range(NQC):
                q0 = j4 * 512
                # filler PE work for this chunk's two attention blocks:
                # chunk j4+1's qkv tiles + chunk j4-1's projection tiles.
                fillers = []
                if j4 + 1 < NQC:
                    tn = j4 + 1
                    fillers.append(lambda tn=tn: qk_tile(wq_sb, qT_sb, 0, tn))
                    fillers.append(lambda tn=tn: qk_tile(wk_sb, kT_sb, 0, tn))
                    for tt in range(4 * tn, 4 * tn + 4):
                        fillers.append(lambda tt=tt: v_tile(tt))
                    fillers.append(lambda tn=tn: qk_tile(wq_sb, qT_sb, 1, tn))
                    fillers.append(lambda tn=tn: qk_tile(wk_sb, kT_sb, 1, tn))
                # projection fillers shifted later: chunks 0-1 are PE-rich
                # while chunk 3 is ACT-paced with spare PE slots, so each
                # chunk takes the later half of proj(j4-1) plus the earlier
                # half of proj(j4-2).
                if j4 >= 2:
                    for k in range(4, 8):
                        fillers.append(
                            lambda k=k, j4=j4: proj_tile((j4 - 2) * 8 + k))
                if j4 > 0:
                    for k in range(4):
                        fillers.append(
                            lambda k=k, j4=j4: proj_tile((j4 - 1) * 8 + k))
                if j4 == NQC - 1:
                    for k in range(4, 8):
                        fillers.append(
                            lambda k=k: proj_tile((NQC - 2) * 8 + k))
                fillers.reverse()  # pop() order = emission order above

                # last chunk: first head-pair's half of its own projection
                # runs as hp=1 filler (yT for hp=0 is ready then); the other
                # half + its DMA are all that remain after the last exp.
                fillers_hp1 = []
                if j4 == NQC - 1:
                    for k in range(8):
                        fillers_hp1.append(
                            lambda k=k: proj_tile(
                                (NQC - 1) * 8 + k, ccs=(0,), evict="vector")
                        )
                fillers_hp1.reverse()

                nsteps = 2 * 4 * (j4 + 1)
                fill_done = 0
                fill_total = len(fillers)

                for hp in range(2):
                    # two heads interleaved per k-tile: one shared 2-bank
                    # S tile, one wide exp for both heads, two independent
                    # y accumulations.
                    h0, h1 = 2 * hp, 2 * hp + 1
                    ch = hp
                    y0 = y_pool.tile([P, 512], F32, name="yps0", tag="yps")
                    y1 = y_pool.tile([P, 512], F32, name="yps1", tag="yps")
                    nk = 4 * (j4 + 1)
                    # diagonal k-tiles first: their exp->mask->PV chains
                    # overlap the full tiles, and the last PV (stop) has no
                    # gpsimd mask on its critical path.
                    iorder = list(range(4 * j4, nk)) + list(range(0, 4 * j4))
                    p2s = {}
                    for iidx, i in enumerate(iorder):
                        m0 = max(0, i - 4 * j4)
                        c0 = P * m0
                        s2 = s_pool.tile([P, 1024], F32, name="sps", tag="sps")
                        for half, po in ((0, 0), (1, 64)):
                            nc.tensor.matmul(
                                s2[:, half * 512 + c0: half * 512 + 512],
                                kT_sb[po:po + D, ch * T + i * P: ch * T + i * P + P],
                                qT_sb[po:po + D, ch * T + q0 + c0: ch * T + q0 + 512],
                                start=True,
                                stop=True,
                            )
                        p2 = att.tile([P, 1024], BF16, name="pt", tag="pt")
                        p2s[i] = p2
                        # one exp spanning both heads' live columns; for
                        # diagonal tiles the dead middle zone [512, 512+c0)
                        # is exp'd too (unwritten PSUM -> garbage) but never
                        # read downstream.
                        nc.scalar.activation(
                            p2[:, c0:1024], s2[:, c0:1024], Exp, scale=0.125
                        )
                        if m0 > 0 or i == 4 * j4:
                            for half in range(2):
                                nc.gpsimd.tensor_mul(
                                    p2[:, half * 512 + c0: half * 512 + c0 + P],
                                    p2[:, half * 512 + c0: half * 512 + c0 + P],
                                    mk_sb[:, :],
                                )
                        # PV lags S by two steps so exp(i) has extra slack
                        # before the PE queue reaches PV(i).
                        if iidx >= 2:
                            ip = iorder[iidx - 2]
                            m0p = max(0, ip - 4 * j4)
                            c0p = P * m0p
                            p2p = p2s.pop(ip)
                            for half, y_ps, hh in ((0, y0, h0), (1, y1, h1)):
                                nc.tensor.matmul(
                                    y_ps[0:VA, c0p:512],
                                    va_sb[:, (ip * HL + hh) * VA:(ip * HL + hh) * VA + VA],
                                    p2p[:, half * 512 + c0p: half * 512 + 512],
                                    start=(iidx == 2),
                                    stop=False,
                                )
                        # filler: keep the PE queue dense while ACT catches
                        # up; distribute evenly across this chunk's steps.
                        step_no = hp * nk + iidx + 1
                        want = fill_total * step_no // nsteps
                        while fill_done < want:
                            fillers.pop()()
                            fill_done += 1
                        if hp == 1 and fillers_hp1:
                            fillers_hp1.pop()()
                    # drain the two pending PV pairs
                    for tail_i, ip in enumerate(iorder[-2:]):
                        m0p = max(0, ip - 4 * j4)
                        c0p = P * m0p
                        p2p = p2s.pop(ip)
                        for half, y_ps, hh in ((0, y0, h0), (1, y1, h1)):
                            nc.tensor.matmul(
                                y_ps[0:VA, c0p:512],
                                va_sb[:, (ip * HL + hh) * VA:(ip * HL + hh) * VA + VA],
                                p2p[:, half * 512 + c0p: half * 512 + 512],
                                start=False,
                                stop=(tail_i == 1),
                            )

                    if j4 == NQC - 1:
                        # keep the PE busy through the tail-chain latency so
                        # the HAM clock gate stays at 8/8 for the final
                        # projection burst.  Fresh s-pool tile: its slot is
                        # free once the last exps have drained (warm_ps's
                        # slot was recycled into live y accumulators).
                        s_dummy = s_pool.tile([P, 1024], F32, name="sdum", tag="sps")
                        for _ in range(12 if hp == 1 else 6):
                            nc.tensor.matmul(
                                s_dummy[:, 0:512], warm_r[:, 0:P], warm_r[:, :],
                                start=True, stop=True,
                            )
                    tails = [(0, y0, h0), (1, y1, h1)]
                    if j4 == NQC - 1 and hp == 1:
                        tails.reverse()  # gating head's short chain first
                    for half, y_ps, hh in tails:
                        po = 64 * half
                        last = (j4 == NQC - 1 and hp == 1)
                        if not last:
                            # evict y rows (frees the PSUM bank), denom row
                            # to a partition-0 tile, approx reciprocal,
                            # DMA-roundtrip broadcast, one DVE multiply.
                            y_sb = misc.tile([P, 512], F32, name="ysb", tag="ysb")
                            nc.vector.tensor_copy(y_sb[0:D, :], y_ps[0:D, :])
                            d_sb = misc.tile([1, 512], F32, name="dsb", tag="dsb")
                            nc.vector.tensor_copy(d_sb[:, :], y_ps[D:VA, :])
                            rc = misc.tile([1, 512], F32, name="rc", tag="rc")
                            nc.vector.reciprocal_approx_fast(rc[:, :], d_sb[:, :])
                            slot = j4 * HL + hh
                            nc.sync.dma_start(
                                out=rc_dram[slot:slot + 1, :], in_=rc[:, :]
                            )
                            bc_sb = misc.tile([D, 512], F32, name="bcsb", tag="bcsb")
                            nc.sync.dma_start(
                                out=bc_sb[:, :],
                                in_=rc_dram[slot:slot + 1, :].to_broadcast((D, 512)),
                            )
                            nc.vector.tensor_mul(
                                yT_sb[po:po + D, ch * T + q0: ch * T + q0 + 512],
                                y_sb[0:D, :],
                                bc_sb[0:D, :],
                            )
                        else:
                            # final block: shortest-latency chains.  ScalarE
                            # is idle after the last exp, so the PSUM reads
                            # go there; DVE keeps recip+mul.
                            d_sb = misc.tile([1, 512], F32, name="dsb", tag="dsb")
                            nc.scalar.copy(d_sb[:, :], y_ps[D:VA, :])
                            rc = misc.tile([1, 512], F32, name="rc", tag="rc")
                            nc.vector.reciprocal_approx_fast(rc[:, :], d_sb[:, :])
                            rcb = misc.tile([1, 512], BF16, name="rcb", tag="rcb")
                            nc.scalar.copy(rcb[:, :], rc[:, :])
                            bc_ps = mm_pool.tile([P, 512], F32, name="bcps", tag="mmps")
                            nc.tensor.matmul(
                                bc_ps[:, :], ones_bf[0:1, :], rcb[:, :],
                                start=True, stop=True,
                            )
                            bc2 = misc.tile([D, 512], F32, name="bcsb", tag="bcsb")
                            nc.scalar.copy(bc2[:, :], bc_ps[0:D, :])
                            nc.vector.tensor_mul(
                                yT_sb[po:po + D, ch * T + q0: ch * T + q0 + 512],
                                y_ps[0:D, :],
                                bc2[:, :],
                            )

            # bridge the tail-chain latency (reciprocal -> broadcast ->
            # multiply, ~2us) with more dummy matmuls: a warm-state HAM
            # window is only ~1.7us, so an idle PE here would re-throttle
            # the final projection burst to half clock.
            s_dummy2 = s_pool.tile([P, 1024], F32, name="sdum2", tag="sps")
            for _ in range(10):
                nc.tensor.matmul(
                    s_dummy2[:, 0:512], warm_r[:, 0:P], warm_r[:, :],
                    start=True, stop=True,
                )
            # final: remaining head-pair half of the last chunk's
            # projection.  Evictions split across DVE+ScalarE (both idle);
            # one contiguous 512KB DMA per 128-row stripe.
            for tt in range(4 * (NQC - 1), 4 * NQC):
                trow = tt * P
                prs = []
                for ocn in range(2):
                    pr_ps = mm_pool.tile([P, 512], F32, name="prps", tag="mmps")
                    nc.tensor.matmul(
                        pr_ps[:, :],
                        yT_sb[:, T + trow: T + trow + P],
                        wp_sb[:, C + ocn * 512: C + ocn * 512 + 512],
                        start=True,
                        stop=True,
                    )
                    prs.append(pr_ps)
                o_sb = outp.tile([P, 2 * 512], BF16, name="osb2", tag="osb2")
                nc.vector.tensor_copy(o_sb[:, 0:512], prs[0][:, :])
                nc.scalar.copy(o_sb[:, 512:1024], prs[1][:, :])
                nc.sync.dma_start(
                    out=out_b[trow - (NQC - 1) * 512: trow - (NQC - 1) * 512 + P, :],
                    in_=o_sb[:, :],
                )
    nc.compile()
    return nc


def _get_nc():
    global _nc_cache
    if _nc_cache is None:
        _nc_cache = _build_nc()
    return _nc_cache


def _prepare_in_maps(x, W_qkv, W_proj):
    x = np.asarray(x, np.float32)
    W_qkv = np.asarray(W_qkv, np.float32)
    W_proj = np.asarray(W_proj, np.float32)
    # [r, j] = 1 where j >= r (upper triangle incl diag, in S^T [k, q] layout)
    tri = (np.arange(P)[None, :] >= np.arange(P)[:, None]).astype(np.float32)
    tri = tri.astype(_BF16)
    in_maps = []
    for c in range(8):
        b, g = c // 4, c % 4
        r0 = OL * g
        def img(a):
            # [R, Y] with R = n*128 rows -> SBUF image [128, n*Y]
            n = a.shape[0] // P
            return np.ascontiguousarray(
                a.reshape(n, P, a.shape[1]).transpose(1, 0, 2).reshape(P, -1)
            ).astype(_BF16)

        xt = x[b].T  # [C, T]
        # group columns by 512-query chunk: [128, (tch, kc, 512)]
        xt_img = (
            xt.reshape(KC, P, NQC, 512)
            .transpose(1, 2, 0, 3)
            .reshape(P, -1)
        )
        in_maps.append({
            "xT": np.ascontiguousarray(xt_img).astype(_BF16),
            "wqT": img(W_qkv[r0:r0 + OL, :].T),
            "wkT": img(W_qkv[C + r0:C + r0 + OL, :].T),
            "wvT": img(W_qkv[2 * C + r0:2 * C + r0 + OL, :].T),
            "wpT": img(W_proj[:, r0:r0 + OL].T),
            "mask_tri": tri,
        })
    return in_maps


def _combine(results):
    out = np.zeros((B, T, C), np.float32)
    for c in range(8):
        out[c // 4] += results[c]["out"].astype(np.float32)
        out[c // 4][(NQC - 1) * 512:] += results[c]["out_b"].astype(np.float32)
    return out


def kernel(x, W_qkv, W_proj):
    nc = _get_nc()
    in_maps = _prepare_in_maps(x, W_qkv, W_proj)
    try:
        res = bass_utils.run_bass_kernel_spmd(nc, in_maps, core_ids=list(range(8)))
    except Exception:
        # rare transient NRT device errors; one retry
        res = bass_utils.run_bass_kernel_spmd(nc, in_maps, core_ids=list(range(8)))
    return _combine(res.results)


def kernel_traced(x, W_qkv, W_proj, trace_cores=None):
    """Like kernel() but returns (out, exec_time_ns) using an NTFF profile."""
    nc = _get_nc()
    in_maps = _prepare_in_maps(x, W_qkv, W_proj)
    res = bass_utils.run_bass_kernel_spmd(
        nc, in_maps, core_ids=list(range(8)), trace=True, trace_cores=trace_cores
    )
    return _combine(res.results), res.exec_time_ns
